# revision 35
# baseline (speedup 1.0000x reference)
"""Trainium2 Bass kernel for nn_Attention: per-head QKV attention + out-proj.

Contract: kernel(**inputs) takes FULL unsharded inputs
  x [8, 1024, 768] f32, Wqkv [12, 768, 192] f32, bqkv [12, 192] f32,
  Wo [768, 768] f32, bo [768] f32
returns FULL output [8, 1024, 768] f32.

Strategy: pure data-parallel over batch (8 batches -> 8 NeuronCores), no
collectives.  Each core computes its batch end-to-end in bf16 matmuls.

Math notes:
  - softmax rows sum to 1 => attn @ (v + bv) = attn @ v + bv, and since the
    attention output is immediately projected, bv folds into the projection
    bias: bo2 = bo + concat(bv) @ Wo.  V-bias never touches the device.
  - softmax is computed unnormalized (scores*SCALE are O(1), exp never
    overflows); the denominator r is produced by the SAME PV matmul via a
    ones-column appended to V (row 64 of the PV psum accumulator), then
    divided out with a fast Newton reciprocal + DRAM-bounce broadcast.

Schedule notes: the head-pair loop interleaves QKV projection of pair t+1
with scores/exp/PV of pair t at Sk-chunk granularity, so TensorE keeps
dense work while ScalarE burns through the exps (which are the pacing
engine) -- this also keeps the PE HAM clock-gate warm (2.4 GHz).
"""

import math
import os

import numpy as np
import ml_dtypes

import concourse.bass as bass
import concourse.tile as tile
from concourse import bacc, mybir
from concourse.bass_utils import run_bass_kernel_spmd
from concourse.tile_rust import add_dep_helper

B, S, D, H, HD = 8, 1024, 768, 12, 64
SCALE = 1.0 / math.sqrt(D)
FP = mybir.dt.float32
BF = mybir.dt.bfloat16
KC = D // 128   # 6 contraction chunks
SC = S // 128   # 8 seq chunks
NQ = S // 512   # 2 free-dim chunks of 512

AluOp = mybir.AluOpType
ActFn = mybir.ActivationFunctionType

# Results of the last hardware run (for test harness introspection).
last_results = None


def _build_kernel_body(tc, out_d, x_d, wqk_d, wv_d, wo_d, bqk_d, bo2_d):
    nc = tc.nc
    rscratch_d = nc.dram_tensor("rscratch", [H, S], FP).ap()

    # Chain every TensorE instruction to the previous one with a no-sync
    # ordering edge: the Tile scheduler otherwise reorders the PE stream by
    # modeled readiness, undoing the deliberate scores/PV/QKV interleave
    # that keeps ScalarE (exp) and TensorE both busy.
    _pe_last = [None]

    def _chain(inst):
        if _pe_last[0] is not None:
            add_dep_helper(inst.ins, _pe_last[0].ins, sync=False,
                           reason="pe-order")
        _pe_last[0] = inst
        return inst

    def MM(*a, reuse_w=False, **k):
        inst = nc.tensor.matmul(*a, **k)
        if reuse_w:
            # stationary operand identical to the previous matmul in the
            # chained PE stream: skip the redundant LDWEIGHTS (bf16-safe)
            inst.ins.ldweights = False
        return _chain(inst)

    def TR(*a, **k):
        return _chain(nc.tensor.transpose(*a, **k))

    from contextlib import ExitStack

    with ExitStack() as ctx:
        wpool = ctx.enter_context(tc.tile_pool(name="weights", bufs=1))
        bigs = ctx.enter_context(tc.tile_pool(name="bigs", bufs=1))
        workp = ctx.enter_context(tc.tile_pool(name="work", bufs=2))
        outp = ctx.enter_context(tc.tile_pool(name="outstage", bufs=2))
        # transient psum: transposes / qkT / v / scores / proj share 2 slots
        # of [128, 1024] f32 (2 banks each); pv accumulators get the rest.
        psq = ctx.enter_context(tc.tile_pool(name="ps_t", bufs=2, space="PSUM"))
        pspv = ctx.enter_context(tc.tile_pool(name="ps_pv", bufs=2, space="PSUM"))

        # ---- weight/bias tiles (DMAs issued after the x loads below) ----
        wqk_sb = wpool.tile([128, KC, 2 * D], BF)
        wv_sb = wpool.tile([128, KC, D], BF)
        wo_sb = wpool.tile([128, KC, D], BF)
        bqk_sb = wpool.tile([128, 2 * KC], FP)
        bo_sb = wpool.tile([128, D], FP)

        # ---- x: load+cast (SWDGE), then TensorE-transpose to xT [D, S] ----
        from concourse.masks import make_identity

        ident = wpool.tile([128, 128], BF)
        make_identity(nc, ident[:])
        ones64 = wpool.tile([1, HD], BF)
        nc.vector.memset(ones64[:], 1.0)
        xT = bigs.tile([128, KC, S], BF)
        qkT = bigs.tile([128, 2 * KC, S], BF)
        with tc.tile_pool(name="xf32", bufs=4) as xf32p, \
             tc.tile_pool(name="xstage", bufs=8) as xpool:
            xbf = []
            for sc in range(SC):
                tf = xf32p.tile([128, D], FP, tag="xf", name=f"xf_{sc}")
                nc.scalar.dma_start(tf[:], x_d[sc * 128:(sc + 1) * 128, :])
                t = xpool.tile([128, D], BF, tag="xbf", name=f"xbf_{sc}")
                nc.vector.tensor_copy(t[:], tf[:])
                xbf.append(t)
            # weights on the sync queue (x uses scalar); wqk first -- it is
            # needed ~12us in by pair 0's projections
            nc.sync.dma_start(wqk_sb[:],
                              wqk_d.rearrange("(kc p) f -> p kc f", p=128))
            nc.sync.dma_start(wv_sb[:],
                              wv_d.rearrange("(kc p) f -> p kc f", p=128))
            nc.sync.dma_start(wo_sb[:],
                              wo_d.rearrange("(kc p) f -> p kc f", p=128))
            nc.sync.dma_start(bqk_sb[:],
                              bqk_d.rearrange("(j p) -> p j", p=128))
            nc.sync.dma_start(
                bo_sb[:],
                bo2_d.rearrange("(a f) -> a f", a=1).partition_broadcast(128),
            )
            # pair 0's q/k projections ride along the transpose loop, one
            # kc-chunk behind; their accumulators hold the two psq slots, so
            # the transposes borrow the (idle) pv pool for psum
            qk_ps = {m: psq.tile([128, S], FP, tag="ps", name=f"qk_{m}")
                     for m in (0, KC)}
            for dt_ in range(KC):
                for n in range(NQ):
                    ps = pspv.tile([128, 512], BF, tag="pv", name=f"tr_{dt_}_{n}")
                    for j in range(4):
                        sc = 4 * n + j
                        TR(
                            ps[:, j * 128:(j + 1) * 128],
                            xbf[sc][:, dt_ * 128:(dt_ + 1) * 128],
                            ident[:],
                        )
                    nc.vector.tensor_copy(
                        xT[:, dt_, n * 512:(n + 1) * 512], ps[:]
                    )
                for m in (0, KC):
                    lhsT = wqk_sb[:, dt_, m * 128:(m + 1) * 128]
                    for nn in range(NQ):
                        MM(
                            qk_ps[m][:, nn * 512:(nn + 1) * 512],
                            lhsT,
                            xT[:, dt_, nn * 512:(nn + 1) * 512],
                            start=(dt_ == 0),
                            stop=(dt_ == KC - 1),
                            reuse_w=(nn > 0),
                        )
            for m in (0, KC):
                nc.vector.tensor_scalar_add(qkT[:, m, :], qk_ps[m][:],
                                            bqk_sb[:, m:m + 1])

        # et pool created here so it reuses the SBUF released by the x
        # staging pools above (allocator assigns space at pool creation)
        etp = ctx.enter_context(tc.tile_pool(name="et", bufs=4))

        # ---- persistent sbuf tensors ----
        # qkT[:, m, :]: m 0..5 -> qT (heads 2m, 2m+1 on partitions 0:64,
        # 64:128), m 6..11 -> kT likewise (allocated in the startup block).
        vaug = bigs.tile([128, SC, H * (HD + 1)], BF)  # v + ones column per head
        vaug4 = vaug.rearrange("p s (h c) -> p s h c", c=HD + 1)
        nc.vector.memset(vaug4[:, :, :, HD:HD + 1], 1.0)
        outT = bigs.tile([128, KC, S], BF)

        def qkv_m(m):
            """project one 128-col block of q or k (m 0..5 q, 6..11 k)"""
            ps = psq.tile([128, S], FP, tag="ps", name=f"qk_{m}")
            for kc in range(KC):
                lhsT = wqk_sb[:, kc, m * 128:(m + 1) * 128]
                for n in range(NQ):
                    MM(
                        ps[:, n * 512:(n + 1) * 512],
                        lhsT,
                        xT[:, kc, n * 512:(n + 1) * 512],
                        start=(kc == 0),
                        stop=(kc == KC - 1),
                        reuse_w=(n > 0),
                    )
            nc.vector.tensor_scalar_add(qkT[:, m, :], ps[:], bqk_sb[:, m:m + 1])

        def v_chunk(sc):
            ps = psq.tile([128, S], FP, tag="ps", name=f"v_{sc}")
            for kc in range(KC):
                lhsT = xT[:, kc, sc * 128:(sc + 1) * 128]
                MM(ps[:, 0:512], lhsT, wv_sb[:, kc, 0:512],
                   start=(kc == 0), stop=(kc == KC - 1))
                MM(ps[:, 512:D], lhsT, wv_sb[:, kc, 512:D],
                   start=(kc == 0), stop=(kc == KC - 1), reuse_w=True)
            nc.vector.tensor_copy(
                vaug4[:, sc, :, 0:HD],
                ps[:, 0:D].rearrange("p (h c) -> p h c", c=HD),
            )

        def scores_chunk(t, sk, et_pair):
            for h01 in range(2):
                ps = psq.tile([128, S], FP, tag="ps", name=f"sc_{t}_{sk}_{h01}")
                lo, hi = h01 * 64, (h01 + 1) * 64
                lhsT = qkT[lo:hi, KC + t, sk * 128:(sk + 1) * 128]
                for n in range(NQ):
                    MM(
                        ps[:, n * 512:(n + 1) * 512],
                        lhsT,
                        qkT[lo:hi, t, n * 512:(n + 1) * 512],
                        start=True,
                        stop=True,
                        tile_position=(h01 * 64, 0),
                        reuse_w=(n > 0),
                    )
                nc.scalar.activation(
                    et_pair[h01][:, sk, :], ps[:], ActFn.Exp, scale=SCALE
                )

        def pv_chunk(t, sk, et_pair, pv_pair):
            for h01 in range(2):
                h = 2 * t + h01
                for n in range(NQ):
                    MM(
                        pv_pair[h01][:, n * 512:(n + 1) * 512],
                        vaug4[:, sk, h, :],
                        et_pair[h01][:, sk, n * 512:(n + 1) * 512],
                        start=(sk == 0),
                        stop=(sk == SC - 1),
                        reuse_w=(n > 0),
                    )

        def pv_finalize(t, pv_pair):
            for h01 in range(2):
                h = 2 * t + h01
                pv = pv_pair[h01]
                # evacuate the accumulator to SBUF immediately: the psum slot
                # gates the next pair's PV, while the rest of this chain
                # (recip + broadcast DMA bounce) has plenty of slack
                u = workp.tile([HD + 1, S], FP, tag="u", name=f"u_{h}")
                nc.vector.tensor_copy(u[:], pv[:])
                # 1/r: custom-DVE Newton recip (~51 ULP, one op).  The custom
                # uop misbehaves on HW unless the source is SBUF at base
                # partition 0, so stage the row through a partition-0 tile.
                rrow = workp.tile([1, S], FP, tag="rrow", name=f"rrow_{h}")
                nc.vector.tensor_copy(rrow[:], u[HD:HD + 1, :])
                recip = workp.tile([1, S], FP, tag="recip", name=f"recip_{h}")
                nc.vector.reciprocal_approx_fast(recip[:], rrow[:])
                # broadcast along partitions via a DRAM bounce (SBUF DMA
                # sources cannot have partition step 0; DRAM can)
                if t < KC - 1:
                    nc.sync.dma_start(rscratch_d[h, :], recip[0:1, :])
                    rb = workp.tile([HD, S], FP, tag="rb", name=f"rb_{h}")
                    nc.sync.dma_start(
                        rb[:], rscratch_d[h:h + 1, :].partition_broadcast(HD)
                    )
                    nc.vector.tensor_tensor(
                        outT[h01 * 64:(h01 + 1) * 64, t, :],
                        u[0:HD, :],
                        rb[:],
                        op=AluOp.mult,
                    )
                else:
                    recip_bf = workp.tile([1, S], BF, tag="recipbf",
                                          name=f"recipbf_{h}")
                    nc.vector.tensor_copy(recip_bf[:], recip[:])
                    bc = psq.tile([HD, S], FP, tag="ps", name=f"bc_{h}")
                    for n in range(NQ):
                        MM(bc[:, n * 512:(n + 1) * 512], ones64[:],
                           recip_bf[0:1, n * 512:(n + 1) * 512],
                           start=True, stop=True)
                    nc.vector.tensor_tensor(
                        outT[h01 * 64:(h01 + 1) * 64, t, :],
                        u[0:HD, :],
                        bc[:],
                        op=AluOp.mult,
                    )

        # ---- main pipeline ----
        # Flat software pipeline over 48 (pair, sk) chunks: pv(j-2) rides 2
        # chunk-slots behind scores(j), crossing pair boundaries, so neither
        # TensorE nor ScalarE ever drains.  V-projection chunks fill pair 0's
        # exp gaps (pv needs vaug[sk] only at its own sk); q/k projections of
        # pair t+1 are injected mid-pair.
        et_tiles = {}
        pv_tiles = {}

        def emit_pv(j):
            t, sk = j // SC, j % SC
            if sk == 0:
                pv_tiles[t] = [
                    pspv.tile([HD + 1, S], FP, tag="pv", name=f"pv_{2 * t + i}")
                    for i in range(2)
                ]
            pv_chunk(t, sk, et_tiles[t], pv_tiles[t])
            if sk == SC - 1:
                pv_finalize(t, pv_tiles[t])
                del pv_tiles[t], et_tiles[t]

        NCH = KC * SC
        for j in range(NCH):
            t, sk = j // SC, j % SC
            if sk == 0:
                et_tiles[t] = [
                    etp.tile([128, SC, S], BF, tag="et", name=f"et_{t}_{i}")
                    for i in range(2)
                ]
            scores_chunk(t, sk, et_tiles[t])
            if t == 0:
                v_chunk(sk)
            if t + 1 < KC:
                if sk == 3:
                    qkv_m(t + 1)
                elif sk == 4:
                    qkv_m(KC + t + 1)
            if j >= 2:
                emit_pv(j - 2)
        emit_pv(NCH - 2)
        emit_pv(NCH - 1)

        # ---- output projection ----
        for sc in range(SC):
            ps = psq.tile([128, S], FP, tag="ps", name=f"o_{sc}")
            for kc in range(KC):
                lhsT = outT[:, kc, sc * 128:(sc + 1) * 128]
                MM(ps[:, 0:512], lhsT, wo_sb[:, kc, 0:512],
                   start=(kc == 0), stop=(kc == KC - 1))
                MM(ps[:, 512:D], lhsT, wo_sb[:, kc, 512:D],
                   start=(kc == 0), stop=(kc == KC - 1), reuse_w=True)
            osb = outp.tile([128, D], FP, tag="osb", name=f"osb_{sc}")
            nc.vector.tensor_tensor(osb[:], ps[:, 0:D], bo_sb[:], op=AluOp.add)
            (nc.scalar if sc % 2 else nc.sync).dma_start(
                out_d[sc * 128:(sc + 1) * 128, :], osb[:])


def build():
    """Build + compile the per-core Bass module. Returns the Bacc object."""
    nc = bacc.Bacc("TRN2", target_bir_lowering=False, debug=False, num_devices=B)
    x_d = nc.dram_tensor("x", [S, D], FP, kind="ExternalInput").ap()
    wqk_d = nc.dram_tensor("wqk", [D, 2 * D], BF, kind="ExternalInput").ap()
    wv_d = nc.dram_tensor("wv", [D, D], BF, kind="ExternalInput").ap()
    wo_d = nc.dram_tensor("wo", [D, D], BF, kind="ExternalInput").ap()
    bqk_d = nc.dram_tensor("bqk", [2 * D], FP, kind="ExternalInput").ap()
    bo2_d = nc.dram_tensor("bo2", [D], FP, kind="ExternalInput").ap()
    out_d = nc.dram_tensor("out", [S, D], FP, kind="ExternalOutput").ap()
    with tile.TileContext(nc) as tc:
        _build_kernel_body(tc, out_d, x_d, wqk_d, wv_d, wo_d, bqk_d, bo2_d)
    nc.compile()
    return nc


def prep_weights(Wqkv, bqkv, Wo, bo):
    """Host-side weight packing (numpy only)."""
    # Wqkv [H, D, 3*HD] -> Wq_all/Wk_all/Wv_all [D, H*HD]
    Wq = np.transpose(Wqkv[:, :, 0:HD], (1, 0, 2)).reshape(D, D)
    Wk = np.transpose(Wqkv[:, :, HD:2 * HD], (1, 0, 2)).reshape(D, D)
    Wv = np.transpose(Wqkv[:, :, 2 * HD:], (1, 0, 2)).reshape(D, D)
    wqk = np.concatenate([Wq, Wk], axis=1)  # [D, 2D]
    bq = bqkv[:, 0:HD].reshape(D)
    bk = bqkv[:, HD:2 * HD].reshape(D)
    bv = bqkv[:, 2 * HD:].reshape(D)
    bqk = np.concatenate([bq, bk])  # [2D]
    bo2 = bo.astype(np.float64) + bv.astype(np.float64) @ Wo.astype(np.float64)
    bf16 = ml_dtypes.bfloat16
    return {
        "wqk": np.ascontiguousarray(wqk.astype(bf16)),
        "wv": np.ascontiguousarray(Wv.astype(bf16)),
        "wo": np.ascontiguousarray(Wo.astype(bf16)),
        "bqk": np.ascontiguousarray(bqk.astype(np.float32)),
        "bo2": np.ascontiguousarray(bo2.astype(np.float32)),
    }


_nc_cache = None


def kernel(x, Wqkv, bqkv, Wo, bo):
    global _nc_cache, last_results
    if _nc_cache is None:
        _nc_cache = build()
    nc = _nc_cache
    w = prep_weights(np.asarray(Wqkv), np.asarray(bqkv), np.asarray(Wo),
                     np.asarray(bo))
    x = np.asarray(x, dtype=np.float32)
    in_maps = [
        {"x": np.ascontiguousarray(x[i]), **w} for i in range(B)
    ]
    res = run_bass_kernel_spmd(
        nc, in_maps, core_ids=list(range(B)),
        trace=bool(os.environ.get("KERNEL_TRACE")),
    )
    last_results = res
    out = np.stack([res.results[i]["out"] for i in range(B)], axis=0)
    return out.astype(np.float32)


# revision 37
# speedup vs baseline: 1.0237x; 1.0237x over previous
"""Trainium2 Bass kernel for nn_Attention: per-head QKV attention + out-proj.

Contract: kernel(**inputs) takes FULL unsharded inputs
  x [8, 1024, 768] f32, Wqkv [12, 768, 192] f32, bqkv [12, 192] f32,
  Wo [768, 768] f32, bo [768] f32
returns FULL output [8, 1024, 768] f32.

Strategy: pure data-parallel over batch (8 batches -> 8 NeuronCores), no
collectives.  Each core computes its batch end-to-end in bf16 matmuls.

Math notes:
  - softmax rows sum to 1 => attn @ (v + bv) = attn @ v + bv, and since the
    attention output is immediately projected, bv folds into the projection
    bias: bo2 = bo + concat(bv) @ Wo.  V-bias never touches the device.
  - softmax is computed unnormalized (scores*SCALE are O(1), exp never
    overflows); the denominator r is produced by the SAME PV matmul via a
    ones-column appended to V (row 64 of the PV psum accumulator), then
    divided out with a fast Newton reciprocal + DRAM-bounce broadcast.

Schedule notes: the head-pair loop interleaves QKV projection of pair t+1
with scores/exp/PV of pair t at Sk-chunk granularity, so TensorE keeps
dense work while ScalarE burns through the exps (which are the pacing
engine) -- this also keeps the PE HAM clock-gate warm (2.4 GHz).
"""

import math
import os

import numpy as np
import ml_dtypes

import concourse.bass as bass
import concourse.tile as tile
from concourse import bacc, mybir
from concourse.bass_utils import run_bass_kernel_spmd
from concourse.tile_rust import add_dep_helper

B, S, D, H, HD = 8, 1024, 768, 12, 64
SCALE = 1.0 / math.sqrt(D)
FP = mybir.dt.float32
BF = mybir.dt.bfloat16
KC = D // 128   # 6 contraction chunks
SC = S // 128   # 8 seq chunks
NQ = S // 512   # 2 free-dim chunks of 512

AluOp = mybir.AluOpType
ActFn = mybir.ActivationFunctionType

# Results of the last hardware run (for test harness introspection).
last_results = None


def _build_kernel_body(tc, out_d, x_d, wqk_d, wv_d, wo_d, bqk_d, bo2_d):
    nc = tc.nc
    rscratch_d = nc.dram_tensor("rscratch", [H, S], FP).ap()

    # Chain every TensorE instruction to the previous one with a no-sync
    # ordering edge: the Tile scheduler otherwise reorders the PE stream by
    # modeled readiness, undoing the deliberate scores/PV/QKV interleave
    # that keeps ScalarE (exp) and TensorE both busy.
    _pe_last = [None]

    def _chain(inst):
        if _pe_last[0] is not None:
            add_dep_helper(inst.ins, _pe_last[0].ins, sync=False,
                           reason="pe-order")
        _pe_last[0] = inst
        return inst

    def MM(*a, reuse_w=False, **k):
        inst = nc.tensor.matmul(*a, **k)
        if reuse_w:
            # stationary operand identical to the previous matmul in the
            # chained PE stream: skip the redundant LDWEIGHTS (bf16-safe)
            inst.ins.ldweights = False
        return _chain(inst)

    def TR(*a, **k):
        return _chain(nc.tensor.transpose(*a, **k))

    from contextlib import ExitStack

    with ExitStack() as ctx:
        wpool = ctx.enter_context(tc.tile_pool(name="weights", bufs=1))
        bigs = ctx.enter_context(tc.tile_pool(name="bigs", bufs=1))
        workp = ctx.enter_context(tc.tile_pool(name="work", bufs=2))
        outp = ctx.enter_context(tc.tile_pool(name="outstage", bufs=2))
        # transient psum: transposes / qkT / v / scores / proj share 2 slots
        # of [128, 1024] f32 (2 banks each); pv accumulators get the rest.
        psq = ctx.enter_context(tc.tile_pool(name="ps_t", bufs=2, space="PSUM"))
        pspv = ctx.enter_context(tc.tile_pool(name="ps_pv", bufs=2, space="PSUM"))

        # ---- weight/bias tiles (DMAs issued after the x loads below) ----
        wqk_sb = wpool.tile([128, KC, 2 * D], BF)
        wv_sb = wpool.tile([128, KC, D], BF)
        wo_sb = wpool.tile([128, KC, D], BF)
        bqk_sb = wpool.tile([128, 2 * KC], FP)
        bo_sb = wpool.tile([128, D], FP)

        # ---- x: load+cast (SWDGE), then TensorE-transpose to xT [D, S] ----
        from concourse.masks import make_identity

        ident = wpool.tile([128, 128], BF)
        make_identity(nc, ident[:])
        ones64 = wpool.tile([1, HD], BF)
        nc.vector.memset(ones64[:], 1.0)
        xT = bigs.tile([128, KC, S], BF)
        qkT = bigs.tile([128, 2 * KC, S], BF)
        with tc.tile_pool(name="xf32", bufs=4) as xf32p, \
             tc.tile_pool(name="xstage", bufs=8) as xpool:
            xbf = []
            for sc in range(SC):
                tf = xf32p.tile([128, D], FP, tag="xf", name=f"xf_{sc}")
                nc.scalar.dma_start(tf[:], x_d[sc * 128:(sc + 1) * 128, :])
                t = xpool.tile([128, D], BF, tag="xbf", name=f"xbf_{sc}")
                nc.vector.tensor_copy(t[:], tf[:])
                xbf.append(t)
            # weights on the sync queue (x uses scalar); wqk first -- it is
            # needed ~12us in by pair 0's projections
            nc.sync.dma_start(wqk_sb[:],
                              wqk_d.rearrange("(kc p) f -> p kc f", p=128))
            nc.sync.dma_start(wv_sb[:],
                              wv_d.rearrange("(kc p) f -> p kc f", p=128))
            nc.sync.dma_start(wo_sb[:],
                              wo_d.rearrange("(kc p) f -> p kc f", p=128))
            nc.sync.dma_start(bqk_sb[:],
                              bqk_d.rearrange("(j p) -> p j", p=128))
            nc.sync.dma_start(
                bo_sb[:],
                bo2_d.rearrange("(a f) -> a f", a=1).partition_broadcast(128),
            )
            # pair 0's q/k projections ride along the transpose loop, one
            # kc-chunk behind; their accumulators hold the two psq slots, so
            # the transposes borrow the (idle) pv pool for psum
            qk_ps = {m: psq.tile([128, S], FP, tag="ps", name=f"qk_{m}")
                     for m in (0, KC)}
            for dt_ in range(KC):
                for n in range(NQ):
                    ps = pspv.tile([128, 512], BF, tag="pv", name=f"tr_{dt_}_{n}")
                    for j in range(4):
                        sc = 4 * n + j
                        TR(
                            ps[:, j * 128:(j + 1) * 128],
                            xbf[sc][:, dt_ * 128:(dt_ + 1) * 128],
                            ident[:],
                        )
                    nc.vector.tensor_copy(
                        xT[:, dt_, n * 512:(n + 1) * 512], ps[:]
                    )
                for m in (0, KC):
                    lhsT = wqk_sb[:, dt_, m * 128:(m + 1) * 128]
                    for nn in range(NQ):
                        MM(
                            qk_ps[m][:, nn * 512:(nn + 1) * 512],
                            lhsT,
                            xT[:, dt_, nn * 512:(nn + 1) * 512],
                            start=(dt_ == 0),
                            stop=(dt_ == KC - 1),
                            reuse_w=(nn > 0),
                        )
            for m in (0, KC):
                nc.vector.tensor_scalar_add(qkT[:, m, :], qk_ps[m][:],
                                            bqk_sb[:, m:m + 1])

        # et pool created here so it reuses the SBUF released by the x
        # staging pools above (allocator assigns space at pool creation)
        etp = ctx.enter_context(tc.tile_pool(name="et", bufs=4))

        # ---- persistent sbuf tensors ----
        # qkT[:, m, :]: m 0..5 -> qT (heads 2m, 2m+1 on partitions 0:64,
        # 64:128), m 6..11 -> kT likewise (allocated in the startup block).
        vaug = bigs.tile([128, SC, H * (HD + 1)], BF)  # v + ones column per head
        vaug4 = vaug.rearrange("p s (h c) -> p s h c", c=HD + 1)
        nc.vector.memset(vaug4[:, :, :, HD:HD + 1], 1.0)
        outT = bigs.tile([128, KC, S], BF)

        def qkv_m(m):
            """project one 128-col block of q or k (m 0..5 q, 6..11 k)"""
            ps = psq.tile([128, S], FP, tag="ps", name=f"qk_{m}")
            for kc in range(KC):
                lhsT = wqk_sb[:, kc, m * 128:(m + 1) * 128]
                for n in range(NQ):
                    MM(
                        ps[:, n * 512:(n + 1) * 512],
                        lhsT,
                        xT[:, kc, n * 512:(n + 1) * 512],
                        start=(kc == 0),
                        stop=(kc == KC - 1),
                        reuse_w=(n > 0),
                    )
            nc.vector.tensor_scalar_add(qkT[:, m, :], ps[:], bqk_sb[:, m:m + 1])

        def v_chunk(sc):
            ps = psq.tile([128, S], FP, tag="ps", name=f"v_{sc}")
            for kc in range(KC):
                lhsT = xT[:, kc, sc * 128:(sc + 1) * 128]
                MM(ps[:, 0:512], lhsT, wv_sb[:, kc, 0:512],
                   start=(kc == 0), stop=(kc == KC - 1))
                MM(ps[:, 512:D], lhsT, wv_sb[:, kc, 512:D],
                   start=(kc == 0), stop=(kc == KC - 1), reuse_w=True)
            nc.vector.tensor_copy(
                vaug4[:, sc, :, 0:HD],
                ps[:, 0:D].rearrange("p (h c) -> p h c", c=HD),
            )

        def scores_chunk(t, sk, et_pair):
            for h01 in range(2):
                ps = psq.tile([128, S], FP, tag="ps", name=f"sc_{t}_{sk}_{h01}")
                lo, hi = h01 * 64, (h01 + 1) * 64
                lhsT = qkT[lo:hi, KC + t, sk * 128:(sk + 1) * 128]
                for n in range(NQ):
                    MM(
                        ps[:, n * 512:(n + 1) * 512],
                        lhsT,
                        qkT[lo:hi, t, n * 512:(n + 1) * 512],
                        start=True,
                        stop=True,
                        tile_position=(h01 * 64, 0),
                        reuse_w=(n > 0),
                    )
                nc.scalar.activation(
                    et_pair[h01][:, sk, :], ps[:], ActFn.Exp, scale=SCALE
                )

        def pv_chunk(t, sk, et_pair, pv_pair):
            for h01 in range(2):
                h = 2 * t + h01
                for n in range(NQ):
                    MM(
                        pv_pair[h01][:, n * 512:(n + 1) * 512],
                        vaug4[:, sk, h, :],
                        et_pair[h01][:, sk, n * 512:(n + 1) * 512],
                        start=(sk == 0),
                        stop=(sk == SC - 1),
                        reuse_w=(n > 0),
                    )

        def pv_finalize(t, pv_pair):
            for h01 in range(2):
                h = 2 * t + h01
                pv = pv_pair[h01]
                # evacuate the accumulator to SBUF immediately: the psum slot
                # gates the next pair's PV, while the rest of this chain
                # (recip + broadcast DMA bounce) has plenty of slack
                u = workp.tile([HD + 1, S], FP, tag="u", name=f"u_{h}")
                nc.vector.tensor_copy(u[:], pv[:])
                # 1/r: custom-DVE Newton recip (~51 ULP, one op).  The custom
                # uop misbehaves on HW unless the source is SBUF at base
                # partition 0, so stage the row through a partition-0 tile.
                rrow = workp.tile([1, S], FP, tag="rrow", name=f"rrow_{h}")
                nc.vector.tensor_copy(rrow[:], u[HD:HD + 1, :])
                recip = workp.tile([1, S], FP, tag="recip", name=f"recip_{h}")
                nc.vector.reciprocal_approx_fast(recip[:], rrow[:])
                # broadcast along partitions via a DRAM bounce (SBUF DMA
                # sources cannot have partition step 0; DRAM can)
                if t < KC - 1:
                    nc.sync.dma_start(rscratch_d[h, :], recip[0:1, :])
                    rb = workp.tile([HD, S], FP, tag="rb", name=f"rb_{h}")
                    nc.sync.dma_start(
                        rb[:], rscratch_d[h:h + 1, :].partition_broadcast(HD)
                    )
                    nc.vector.tensor_tensor(
                        outT[h01 * 64:(h01 + 1) * 64, t, :],
                        u[0:HD, :],
                        rb[:],
                        op=AluOp.mult,
                    )
                else:
                    recip_bf = workp.tile([1, S], BF, tag="recipbf",
                                          name=f"recipbf_{h}")
                    nc.vector.tensor_copy(recip_bf[:], recip[:])
                    bc = psq.tile([HD, S], FP, tag="ps", name=f"bc_{h}")
                    for n in range(NQ):
                        MM(bc[:, n * 512:(n + 1) * 512], ones64[:],
                           recip_bf[0:1, n * 512:(n + 1) * 512],
                           start=True, stop=True)
                    nc.vector.tensor_tensor(
                        outT[h01 * 64:(h01 + 1) * 64, t, :],
                        u[0:HD, :],
                        bc[:],
                        op=AluOp.mult,
                    )

        # ---- main pipeline ----
        # Flat software pipeline over 48 (pair, sk) chunks: pv(j-2) rides 2
        # chunk-slots behind scores(j), crossing pair boundaries, so neither
        # TensorE nor ScalarE ever drains.  V-projection chunks fill pair 0's
        # exp gaps (pv needs vaug[sk] only at its own sk); q/k projections of
        # pair t+1 are injected mid-pair.
        et_tiles = {}
        pv_tiles = {}

        def emit_pv(j):
            t, sk = j // SC, j % SC
            if sk == 0:
                pv_tiles[t] = [
                    pspv.tile([HD + 1, S], FP, tag="pv", name=f"pv_{2 * t + i}")
                    for i in range(2)
                ]
            pv_chunk(t, sk, et_tiles[t], pv_tiles[t])
            if sk == SC - 1:
                pv_finalize(t, pv_tiles[t])
                del pv_tiles[t], et_tiles[t]

        NCH = KC * SC
        for j in range(NCH):
            t, sk = j // SC, j % SC
            if sk == 0:
                et_tiles[t] = [
                    etp.tile([128, SC, S], BF, tag="et", name=f"et_{t}_{i}")
                    for i in range(2)
                ]
            scores_chunk(t, sk, et_tiles[t])
            if t == 0:
                v_chunk(sk)
            if t + 1 < KC:
                if sk == 3:
                    qkv_m(t + 1)
                elif sk == 4:
                    qkv_m(KC + t + 1)
            if j >= 2:
                emit_pv(j - 2)
        emit_pv(NCH - 2)
        emit_pv(NCH - 1)

        # ---- output projection ----
        for sc in range(SC):
            ps = psq.tile([128, S], FP, tag="ps", name=f"o_{sc}")
            for kc in range(KC):
                lhsT = outT[:, kc, sc * 128:(sc + 1) * 128]
                MM(ps[:, 0:512], lhsT, wo_sb[:, kc, 0:512],
                   start=(kc == 0), stop=(kc == KC - 1))
                MM(ps[:, 512:D], lhsT, wo_sb[:, kc, 512:D],
                   start=(kc == 0), stop=(kc == KC - 1), reuse_w=True)
            osb = outp.tile([128, D], FP, tag="osb", name=f"osb_{sc}")
            nc.vector.tensor_tensor(osb[:], ps[:, 0:D], bo_sb[:], op=AluOp.add)
            (nc.scalar if sc % 2 else nc.sync).dma_start(
                out_d[sc * 128:(sc + 1) * 128, :], osb[:])


def build():
    """Build + compile the per-core Bass module. Returns the Bacc object."""
    nc = bacc.Bacc("TRN2", target_bir_lowering=False, debug=False, num_devices=B)
    x_d = nc.dram_tensor("x", [S, D], FP, kind="ExternalInput").ap()
    wqk_d = nc.dram_tensor("wqk", [D, 2 * D], BF, kind="ExternalInput").ap()
    wv_d = nc.dram_tensor("wv", [D, D], BF, kind="ExternalInput").ap()
    wo_d = nc.dram_tensor("wo", [D, D], BF, kind="ExternalInput").ap()
    bqk_d = nc.dram_tensor("bqk", [2 * D], FP, kind="ExternalInput").ap()
    bo2_d = nc.dram_tensor("bo2", [D], FP, kind="ExternalInput").ap()
    out_d = nc.dram_tensor("out", [S, D], FP, kind="ExternalOutput").ap()
    with tile.TileContext(nc) as tc:
        _build_kernel_body(tc, out_d, x_d, wqk_d, wv_d, wo_d, bqk_d, bo2_d)
    nc.compile()
    return nc


def prep_weights(Wqkv, bqkv, Wo, bo):
    """Host-side weight packing (numpy only)."""
    # Wqkv [H, D, 3*HD] -> Wq_all/Wk_all/Wv_all [D, H*HD]
    Wq = np.transpose(Wqkv[:, :, 0:HD], (1, 0, 2)).reshape(D, D)
    Wk = np.transpose(Wqkv[:, :, HD:2 * HD], (1, 0, 2)).reshape(D, D)
    Wv = np.transpose(Wqkv[:, :, 2 * HD:], (1, 0, 2)).reshape(D, D)
    wqk = np.concatenate([Wq, Wk], axis=1)  # [D, 2D]
    bq = bqkv[:, 0:HD].reshape(D)
    bk = bqkv[:, HD:2 * HD].reshape(D)
    bv = bqkv[:, 2 * HD:].reshape(D)
    bqk = np.concatenate([bq, bk])  # [2D]
    bo2 = bo.astype(np.float64) + bv.astype(np.float64) @ Wo.astype(np.float64)
    bf16 = ml_dtypes.bfloat16
    return {
        "wqk": np.ascontiguousarray(wqk.astype(bf16)),
        "wv": np.ascontiguousarray(Wv.astype(bf16)),
        "wo": np.ascontiguousarray(Wo.astype(bf16)),
        "bqk": np.ascontiguousarray(bqk.astype(np.float32)),
        "bo2": np.ascontiguousarray(bo2.astype(np.float32)),
    }


_nc_cache = None


def kernel(x, Wqkv, bqkv, Wo, bo):
    global _nc_cache, last_results
    if _nc_cache is None:
        _nc_cache = build()
    nc = _nc_cache
    w = prep_weights(np.asarray(Wqkv), np.asarray(bqkv), np.asarray(Wo),
                     np.asarray(bo))
    x = np.asarray(x, dtype=np.float32)
    in_maps = [
        {"x": np.ascontiguousarray(x[i]), **w} for i in range(B)
    ]
    res = run_bass_kernel_spmd(
        nc, in_maps, core_ids=list(range(B)),
        trace=bool(os.environ.get("KERNEL_TRACE")),
    )
    last_results = res
    out = np.stack([res.results[i]["out"] for i in range(B)], axis=0)
    return out.astype(np.float32)
